# revision 1
# baseline (speedup 1.0000x reference)
"""Trainium2 Bass kernel for nn_Decoder_30777735643309.

GRU decoder: ses = tanh(lin1(ses_encoding)); 50 sequential GRU steps with
hidden input concat(h, ses); per-step logits over a 10004 vocab.

Strategy (8 cores, no collectives): data-parallel over batch (16 rows/core).
Everything on-chip runs in a transposed layout (features on partitions,
batch in the free dim) so the tiny per-core batch does not waste engine
lanes. Key algebraic split: Hfull = [h, ses] with ses constant across steps,
so gh = h @ Whh[:, :H].T + (ses @ Whh[:, H:].T) — the second term is
precomputed once, halving the recurrent matmul. Input-side gx = emb[x] @
Wih.T and the output logits matmul are batched outside the recurrence.
Matmul inputs are bf16 (fp32 PSUM accumulation).
"""

import numpy as np
import ml_dtypes

import concourse.bacc as bacc
import concourse.mybir as mybir
import concourse.tile as tile
from concourse.bass import IndirectOffsetOnAxis
from concourse.bass_utils import run_bass_kernel_spmd
from concourse.masks import make_identity

F32 = mybir.dt.float32
BF16 = mybir.dt.bfloat16
I32 = mybir.dt.int32
AF = mybir.ActivationFunctionType
OP = mybir.AluOpType

V = 10004
E = 300
EP = 384          # E padded to 3 K-chunks of 128
SH = 1024
H = 512
G = 1024          # GRU hidden = 2*H
G3 = 3 * G        # 3072
B, T = 128, 50
NCORES = 8
BL = B // NCORES  # 16 batch rows per core
NT = T * BL       # 800 (t-major columns: col = t*BL + b)
NTP = 896         # NT padded to 7 chunks of 128
KH = H // 128     # 4 K-chunks for the h-part matmul
M3 = G3 // 128    # 24 feature chunks of the gate dim
MRZ = 2 * G // 128  # 16 chunks for r,z
NB = 2            # column blocking for the batched matmuls
NBW = NTP // NB   # 448 columns per block


def build_program(reps: int = 1, debug: bool = False, loop: bool = False):
    nc = bacc.Bacc()

    # ---- DRAM I/O ----
    d_sesenc = nc.dram_tensor("sesenc", [SH, BL], BF16, kind="ExternalInput")
    d_xw = nc.dram_tensor("xw", [128, 7], I32, kind="ExternalInput")
    d_emb = nc.dram_tensor("emb", [V, E], F32, kind="ExternalInput")
    d_whh_h = nc.dram_tensor("whh_h", [H, G3], BF16, kind="ExternalInput")
    d_whh_s = nc.dram_tensor("whh_s", [H, G3], BF16, kind="ExternalInput")
    d_wih = nc.dram_tensor("wih", [EP, G3], BF16, kind="ExternalInput")
    d_w1 = nc.dram_tensor("w1", [SH, H], BF16, kind="ExternalInput")
    d_w2 = nc.dram_tensor("w2", [G, E], BF16, kind="ExternalInput")
    d_wout = nc.dram_tensor("wout", [EP, V], BF16, kind="ExternalInput")
    d_b1 = nc.dram_tensor("b1t", [128, H // 128], F32, kind="ExternalInput")
    d_bih = nc.dram_tensor("biht", [128, M3], F32, kind="ExternalInput")
    d_bhh = nc.dram_tensor("bhht", [128, M3], F32, kind="ExternalInput")
    d_b2 = nc.dram_tensor("b2t", [128, EP // 128], F32, kind="ExternalInput")
    # t-major rows (row = t*BL + b); host reorders to [BL, T, V]
    d_out = nc.dram_tensor("out", [NT, V], F32, kind="ExternalOutput")

    with tile.TileContext(nc) as tc:
        import contextlib
        with contextlib.ExitStack() as ctx:
            persist = ctx.enter_context(tc.tile_pool(name="persist", bufs=1))
            step = ctx.enter_context(tc.tile_pool(name="step", bufs=3))
            psG = ctx.enter_context(tc.tile_pool(name="psG", bufs=2, space="PSUM"))
            psMM = ctx.enter_context(tc.tile_pool(name="psMM", bufs=4, space="PSUM"))
            psT = ctx.enter_context(tc.tile_pool(name="psT", bufs=2, space="PSUM"))
            woutp = ctx.enter_context(tc.tile_pool(name="woutp", bufs=4))
            lout = ctx.enter_context(tc.tile_pool(name="lout", bufs=4))

            # persistent SBUF tensors
            whh_h_sb = persist.tile([128, KH, G3], BF16)
            w2_sb = persist.tile([128, G // 128, E], BF16)
            gx_sb = persist.tile([128, M3, NTP], BF16)
            embxT = persist.tile([128, EP // 128, NTP], BF16)
            hnT = persist.tile([128, G // 128, NTP], BF16)
            oT = persist.tile([128, EP // 128, NTP], BF16)
            ct = persist.tile([128, M3, BL], F32)
            dt_ = persist.tile([128, M3, BL], F32)
            sesT = persist.tile([128, KH, BL], F32)
            sesT_bf = persist.tile([128, KH, BL], BF16)
            hT = persist.tile([128, KH, BL], BF16)
            b1t = persist.tile([128, H // 128], F32)
            biht = persist.tile([128, M3], F32)
            bhht = persist.tile([128, M3], F32)
            b2t = persist.tile([128, EP // 128], F32)
            xw = persist.tile([128, 7], I32)
            ident = persist.tile([128, 128], F32)

            nc.sync.dma_start(out=whh_h_sb, in_=d_whh_h[:, :].rearrange("(k p) c -> p k c", p=128))
            nc.sync.dma_start(out=w2_sb, in_=d_w2[:, :].rearrange("(k p) c -> p k c", p=128))
            nc.sync.dma_start(out=b1t, in_=d_b1[:, :])
            nc.sync.dma_start(out=biht, in_=d_bih[:, :])
            nc.sync.dma_start(out=bhht, in_=d_bhh[:, :])
            nc.sync.dma_start(out=b2t, in_=d_b2[:, :])
            nc.sync.dma_start(out=xw, in_=d_xw[:, :])
            make_identity(nc, ident)

            import contextlib as _ctxlib

            if loop:
                loop_cm = tc.For_i(0, reps, 1)
                rep_iter = [0]
            else:
                loop_cm = _ctxlib.nullcontext()
                rep_iter = range(reps)

            with loop_cm:
              for _rep in rep_iter:
                with tc.tile_pool(name=f"setup_{_rep}", bufs=1) as setup, \
                     tc.tile_pool(name=f"gatherp_{_rep}", bufs=3) as gatherp:
                    whh_s_sb = setup.tile([128, KH, G3], BF16)
                    w1_sb = setup.tile([128, SH // 128, H], BF16)
                    wih_sb = setup.tile([128, EP // 128, G3], BF16)
                    sesenc_sb = setup.tile([128, SH // 128, BL], BF16)
                    nc.sync.dma_start(out=whh_s_sb, in_=d_whh_s[:, :].rearrange("(k p) c -> p k c", p=128))
                    nc.sync.dma_start(out=w1_sb, in_=d_w1[:, :].rearrange("(k p) c -> p k c", p=128))
                    nc.sync.dma_start(out=wih_sb, in_=d_wih[:, :].rearrange("(k p) c -> p k c", p=128))
                    nc.sync.dma_start(out=sesenc_sb, in_=d_sesenc[:, :].rearrange("(k p) c -> p k c", p=128))

                    # zero the padded staging tensors (pad rows must be 0)
                    nc.vector.memset(embxT, 0.0)
                    nc.vector.memset(oT, 0.0)

                    # ses = tanh(W1 @ ses_encT + b1)  -> [H, BL] as [128, 4, BL]
                    ps_s = psT.tile([128, KH, BL], F32, tag="tp")
                    for m in range(KH):
                        for k in range(SH // 128):
                            nc.tensor.matmul(
                                out=ps_s[:, m, :],
                                lhsT=w1_sb[:, k, m * 128:(m + 1) * 128],
                                rhs=sesenc_sb[:, k, :],
                                start=(k == 0), stop=(k == SH // 128 - 1))
                    for m in range(KH):
                        nc.scalar.activation(sesT[:, m, :], ps_s[:, m, :], AF.Tanh,
                                             bias=b1t[:, m:m + 1])
                    nc.vector.tensor_copy(sesT_bf, sesT)

                    # CT = Whh_ses @ sesT + bhh  -> [3G, BL]
                    ps_gs = psT.tile([128, M3, BL], F32, tag="tp")
                    for m in range(M3):
                        for k in range(KH):
                            nc.tensor.matmul(
                                out=ps_gs[:, m, :],
                                lhsT=whh_s_sb[:, k, m * 128:(m + 1) * 128],
                                rhs=sesT_bf[:, k, :],
                                start=(k == 0), stop=(k == KH - 1))
                    nc.vector.tensor_tensor(
                        out=ct, in0=ps_gs,
                        in1=bhht[:, :, None].broadcast_to([128, M3, BL]), op=OP.add)
                    # DT: rz chunks get CT+bih; n chunks get bih only
                    nc.vector.tensor_tensor(
                        out=dt_[:, 0:MRZ, :], in0=ct[:, 0:MRZ, :],
                        in1=biht[:, 0:MRZ, None].broadcast_to([128, MRZ, BL]), op=OP.add)
                    nc.vector.tensor_copy(
                        dt_[:, MRZ:M3, :],
                        biht[:, MRZ:M3, None].broadcast_to([128, M3 - MRZ, BL]))

                    # gather emb[x] -> transpose -> embxT [EP, NT] bf16
                    for c in range(7):
                        pm = 128 if c < 6 else NT - 6 * 128
                        embx_c = gatherp.tile([128, E], F32, tag="gx")
                        nc.gpsimd.indirect_dma_start(
                            out=embx_c[:pm, :], out_offset=None,
                            in_=d_emb[:, :],
                            in_offset=IndirectOffsetOnAxis(ap=xw[:pm, c:c + 1], axis=0))
                        for k in range(EP // 128):
                            kw = min(128, E - k * 128)
                            if kw <= 0:
                                break
                            ps_t = psT.tile([128, 128], F32, tag="tp")
                            nc.tensor.transpose(
                                out=ps_t[:kw, :pm],
                                in_=embx_c[:pm, k * 128:k * 128 + kw],
                                identity=ident[:pm, :pm])
                            nc.vector.tensor_copy(
                                embxT[:kw, k, c * 128:c * 128 + pm], ps_t[:kw, :pm])

                    # gx = Wih @ embxT (+bih, rz chunks +CT) -> [3G, NTP] bf16
                    for m in range(M3):
                        for nb in range(NB):
                            cs = slice(nb * NBW, (nb + 1) * NBW)
                            ps_gx = psMM.tile([128, NBW], F32, tag="mm")
                            for k in range(EP // 128):
                                nc.tensor.matmul(
                                    out=ps_gx,
                                    lhsT=wih_sb[:, k, m * 128:(m + 1) * 128],
                                    rhs=embxT[:, k, cs],
                                    start=(k == 0), stop=(k == EP // 128 - 1))
                            nc.vector.tensor_tensor(
                                out=gx_sb[:, m, cs].rearrange("p (t b) -> p t b", b=BL),
                                in0=ps_gx[:, :].rearrange("p (t b) -> p t b", b=BL),
                                in1=dt_[:, m, None, :].broadcast_to([128, NBW // BL, BL]),
                                op=OP.add)

                # ---- recurrence ----
                nc.vector.tensor_copy(hT, sesT_bf)
                for t in range(T):
                    ts = slice(t * BL, (t + 1) * BL)
                    ps_g = psG.tile([128, M3, BL], F32, tag="g")
                    for m in range(M3):
                        for k in range(KH):
                            nc.tensor.matmul(
                                out=ps_g[:, m, :],
                                lhsT=whh_h_sb[:, k, m * 128:(m + 1) * 128],
                                rhs=hT[:, k, :],
                                start=(k == 0), stop=(k == KH - 1))
                    rzp = step.tile([128, MRZ, BL], F32, tag="rzp")
                    nc.vector.tensor_tensor(out=rzp, in0=ps_g[:, 0:MRZ, :],
                                            in1=gx_sb[:, 0:MRZ, ts], op=OP.add)
                    rz = step.tile([128, MRZ, BL], F32, tag="rz")
                    nc.scalar.activation(rz, rzp, AF.Sigmoid)
                    ghn = step.tile([128, M3 - MRZ, BL], F32, tag="ghn")
                    nc.vector.tensor_tensor(out=ghn, in0=ps_g[:, MRZ:M3, :],
                                            in1=ct[:, MRZ:M3, :], op=OP.add)
                    t1 = step.tile([128, M3 - MRZ, BL], F32, tag="t1")
                    nc.vector.tensor_tensor(out=t1, in0=rz[:, 0:MRZ // 2, :],
                                            in1=ghn, op=OP.mult)
                    nc.vector.tensor_tensor(out=t1, in0=t1,
                                            in1=gx_sb[:, MRZ:M3, ts], op=OP.add)
                    ntl = step.tile([128, M3 - MRZ, BL], F32, tag="n")
                    nc.scalar.activation(ntl, t1, AF.Tanh)
                    d = step.tile([128, M3 - MRZ, BL], F32, tag="d")
                    nc.vector.tensor_tensor(out=d[:, 0:KH, :], in0=hT,
                                            in1=ntl[:, 0:KH, :], op=OP.subtract)
                    nc.vector.tensor_tensor(out=d[:, KH:, :], in0=sesT,
                                            in1=ntl[:, KH:, :], op=OP.subtract)
                    nc.vector.tensor_tensor(out=d, in0=rz[:, MRZ // 2:, :],
                                            in1=d, op=OP.mult)
                    nc.vector.tensor_tensor(out=hnT[:, :, ts], in0=ntl, in1=d,
                                            op=OP.add)
                    nc.vector.tensor_copy(hT, hnT[:, 0:KH, ts])

                # ---- o = W2 @ hnT + b2 + embx ----
                for m in range(EP // 128):
                    pm = min(128, E - m * 128)
                    for nb in range(NB):
                        cs = slice(nb * NBW, (nb + 1) * NBW)
                        ps_o = psMM.tile([128, NBW], F32, tag="mm")
                        for k in range(G // 128):
                            nc.tensor.matmul(
                                out=ps_o[:pm, :],
                                lhsT=w2_sb[:, k, m * 128:m * 128 + pm],
                                rhs=hnT[:, k, cs],
                                start=(k == 0), stop=(k == G // 128 - 1))
                        tmp_o = step.tile([128, NBW], F32, tag="otmp")
                        nc.vector.tensor_scalar_add(out=tmp_o[:pm, :], in0=ps_o[:pm, :],
                                                    scalar1=b2t[:pm, m:m + 1])
                        nc.vector.tensor_tensor(out=oT[:pm, m, cs], in0=tmp_o[:pm, :],
                                                in1=embxT[:pm, m, cs], op=OP.add)

                if debug and _rep == 0:
                    dbg = {
                        "dbg_ses": ([128, KH * BL], F32, sesT),
                        "dbg_ct": ([128, M3 * BL], F32, ct),
                        "dbg_dt": ([128, M3 * BL], F32, dt_),
                        "dbg_embx": ([128, (EP // 128) * NTP], BF16, embxT),
                        "dbg_gx": ([128, M3 * NTP], BF16, gx_sb),
                        "dbg_hn": ([128, (G // 128) * NTP], BF16, hnT),
                        "dbg_o": ([128, (EP // 128) * NTP], BF16, oT),
                    }
                    for nm, (shp, dt, tl) in dbg.items():
                        dh = nc.dram_tensor(nm, shp, dt, kind="ExternalOutput")
                        nc.sync.dma_start(out=dh[:, :], in_=tl[:, :].rearrange("p a b -> p (a b)"))

                # ---- logits = oT.T @ Wout.T -> DRAM ----
                NV = 20
                for nv in range(NV):
                    nw = min(512, V - nv * 512)
                    wchunk = woutp.tile([128, EP // 128, 512], BF16, tag="w")
                    nc.sync.dma_start(
                        out=wchunk[:, :, :nw],
                        in_=d_wout[:, nv * 512:nv * 512 + nw].rearrange(
                            "(k p) v -> p k v", p=128))
                    for mt in range(7):
                        pm = 128 if mt < 6 else NT - 6 * 128
                        ms = slice(mt * 128, mt * 128 + pm)
                        ps_l = psMM.tile([128, 512], F32, tag="mm")
                        for k in range(EP // 128):
                            nc.tensor.matmul(
                                out=ps_l[:pm, :nw],
                                lhsT=oT[:, k, ms],
                                rhs=wchunk[:, k, :nw],
                                start=(k == 0), stop=(k == EP // 128 - 1))
                        lsb = lout.tile([128, 512], F32, tag="l")
                        if (nv * 7 + mt) % 2 == 0:
                            nc.vector.tensor_copy(lsb[:pm, :nw], ps_l[:pm, :nw])
                        else:
                            nc.scalar.copy(lsb[:pm, :nw], ps_l[:pm, :nw])
                        nc.sync.dma_start(
                            out=d_out[mt * 128:mt * 128 + pm, nv * 512:nv * 512 + nw],
                            in_=lsb[:pm, :nw])

    nc.finalize()
    return nc


_PROG_CACHE = {}


def _get_program(reps: int = 1):
    if reps not in _PROG_CACHE:
        _PROG_CACHE[reps] = build_program(reps)
    return _PROG_CACHE[reps]


def _bf(a):
    return np.ascontiguousarray(a).astype(ml_dtypes.bfloat16)


def _prep_shared(inputs):
    emb = np.ascontiguousarray(inputs["emb"], dtype=np.float32)
    Wih = np.asarray(inputs["Wih"], dtype=np.float32)
    Whh = np.asarray(inputs["Whh"], dtype=np.float32)
    W1 = np.asarray(inputs["W1"], dtype=np.float32)
    W2 = np.asarray(inputs["W2"], dtype=np.float32)
    Wout = np.asarray(inputs["Wout"], dtype=np.float32)

    WhhT = Whh.T  # [G, 3G]
    wih_p = np.zeros((EP, G3), np.float32)
    wih_p[:E] = Wih.T
    wout_p = np.zeros((EP, V), np.float32)
    wout_p[:E] = Wout.T
    b2_p = np.zeros(EP, np.float32)
    b2_p[:E] = np.asarray(inputs["b2"], dtype=np.float32)

    return {
        "emb": emb,
        "whh_h": _bf(WhhT[:H]),
        "whh_s": _bf(WhhT[H:]),
        "wih": _bf(wih_p),
        "w1": _bf(W1.T),
        "w2": _bf(W2.T),
        "wout": _bf(wout_p),
        "b1t": np.ascontiguousarray(
            np.asarray(inputs["b1"], np.float32).reshape(H // 128, 128).T),
        "biht": np.ascontiguousarray(
            np.asarray(inputs["bih"], np.float32).reshape(M3, 128).T),
        "bhht": np.ascontiguousarray(
            np.asarray(inputs["bhh"], np.float32).reshape(M3, 128).T),
        "b2t": np.ascontiguousarray(b2_p.reshape(EP // 128, 128).T),
    }


def make_in_maps(inputs):
    shared = _prep_shared(inputs)
    x = np.asarray(inputs["x"]).astype(np.int32)          # [B, T]
    ses = np.asarray(inputs["ses_encoding"], np.float32)[0]  # [B, SH]
    in_maps = []
    for c in range(NCORES):
        bs = slice(c * BL, (c + 1) * BL)
        xf = np.zeros(NTP, np.int32)
        xf[:NT] = x[bs].T.reshape(-1)  # t-major
        m = dict(shared)
        m["xw"] = np.ascontiguousarray(xf.reshape(7, 128).T)
        m["sesenc"] = _bf(ses[bs].T)
        in_maps.append(m)
    return in_maps


def run(inputs, reps: int = 1, **kwargs):
    nc = _get_program(reps)
    in_maps = make_in_maps(inputs)
    res = run_bass_kernel_spmd(nc, in_maps, core_ids=list(range(NCORES)), **kwargs)
    out = np.concatenate(
        [res.results[c]["out"].reshape(T, BL, V).transpose(1, 0, 2)
         for c in range(NCORES)], axis=0)
    return np.ascontiguousarray(out)


def kernel(**inputs) -> np.ndarray:
    return run(inputs)

